# revision 15
# baseline (speedup 1.0000x reference)
"""DeepSigNet Trainium2 kernel (8-core data-parallel).

Math (per batch element, matching the reference):
  path = tanh(conv1d(x[:64], w, k=3, pad=1) + b).T          # [L=512, 64]
  dx[t] = path[t+1] - path[t], t = 0..510
  S[m, j] = sum_t (path[t, m]) * dx[t, j]   (uncentered)
  The reference uses prefix = path[t] - path[0], i.e.
  S'[m, j] = S[m, j] - p0[m] * lvl1[j]  with p0 = path[0], lvl1 = path[511]-path[0].
  Only the antisymmetric part of S' feeds the MLP (via triu of A = 0.5(S'-S'^T)),
  so any symmetric difference is free: we compute
  S'' = S + p511 (x) p0, which equals S' modulo a symmetric matrix.
  The fc1 weights are host-permuted so that fc1 consumes
  [vec-ish(S'') | lvl1 | pooled | static | 1] directly.

Device layout (per core, 16 batch elems):
  FT [128, 576] sbuf: 36 K-tiles of 16 columns (one col per elem).
    tiles 0..31: S''-features.  FT[p<64, 16t+b] = S''[t, p]
                 FT[p>=64, 16t+b] = S''[32+t, p-64]
    tile 32: p<64 lvl1[p]; p=64 pooled; p>=65 static chan p
    tile 33/34: static chan 128+p / 256+p
    tile 35: p<64 static chan 384+p; p=64 const 1 (biases); p>=65 zero pad
  MLP: out = relu(relu(featT.T @ W1T) @ ...) with PE transposes between layers.
"""

import os
import numpy as np

B, C_IN, C_OUT, L = 128, 64, 64, 512
POST, HID, OUT_DIM = 384, 1024, 128
NCORES = 8
BPC = B // NCORES  # 16
NT1 = 36           # fc1 K-tiles
D1 = NT1 * 128     # 4608 padded fc1 input dim

# float32r runs 1 cycle/row (vs 4 for fp32) on matmuls with N >= 256.
USE_F32R = os.environ.get("DSN_F32R", "1") == "1"
# bf16 fc1/fc2 weights+activations: halves the dominant weight DMA.
W_BF16 = os.environ.get("DSN_WDT", "f32") == "bf16"

_prog_cache = {}


def _build_nc():
    key = ("nc", USE_F32R, W_BF16)
    if key in _prog_cache:
        return _prog_cache[key]

    import concourse.bass as bass
    import concourse.tile as tile
    from concourse import bacc, mybir

    f32 = mybir.dt.float32
    f32r = mybir.dt.float32r
    bf16 = mybir.dt.bfloat16
    wdt = bf16 if W_BF16 else f32
    TANH = mybir.ActivationFunctionType.Tanh

    def mmdt(ap):
        if W_BF16:
            return ap
        return ap.bitcast(f32r) if USE_F32R else ap

    nc = bacc.Bacc(None, target_bir_lowering=False, debug=False)

    x_d = nc.dram_tensor("x", [BPC, C_IN + POST, L], f32, kind="ExternalInput")
    wc_d = nc.dram_tensor("wconv", [128, 192], f32, kind="ExternalInput")
    cb_d = nc.dram_tensor("cbias", [128, 256], f32, kind="ExternalInput")
    w1_d = nc.dram_tensor("w1t", [D1, HID], wdt, kind="ExternalInput")
    w2_d = nc.dram_tensor("w2t", [HID, HID], wdt, kind="ExternalInput")
    b2_d = nc.dram_tensor("b2", [1, HID], f32, kind="ExternalInput")
    w3_d = nc.dram_tensor("w3t", [HID, OUT_DIM], f32, kind="ExternalInput")
    b3_d = nc.dram_tensor("b3", [1, OUT_DIM], f32, kind="ExternalInput")
    idn_d = nc.dram_tensor("idn", [128, 128], f32, kind="ExternalInput")
    cst_d = nc.dram_tensor("csts", [128, 2], f32, kind="ExternalInput")
    t35_d = nc.dram_tensor("t35c", [127, BPC], f32, kind="ExternalInput")
    out_d = nc.dram_tensor("out", [BPC, OUT_DIM], f32, kind="ExternalOutput")

    xa = x_d.ap()
    outa = out_d.ap()

    with tile.TileContext(nc) as tc:
        with (
            tc.tile_pool(name="const", bufs=1) as constp,
            tc.tile_pool(name="ft", bufs=1) as ftp,
            tc.tile_pool(name="ftps", bufs=1, space="PSUM") as ftpsp,
            tc.tile_pool(name="xb", bufs=3) as xbp,
            tc.tile_pool(name="cvps", bufs=2, space="PSUM") as cvpsp,
            tc.tile_pool(name="pt", bufs=2) as ptp,
            tc.tile_pool(name="ptsh", bufs=2) as ptshp,
            tc.tile_pool(name="dd", bufs=2) as ddp,
            tc.tile_pool(name="tmp", bufs=2) as tmpp,
            tc.tile_pool(name="smallps", bufs=2, space="PSUM") as smallps,
            tc.tile_pool(name="prow", bufs=2) as prowp,
            tc.tile_pool(name="xm", bufs=1) as xmp,
            tc.tile_pool(name="wstream", bufs=4) as wsp,
            tc.tile_pool(name="mlpps", bufs=2, space="PSUM") as mlpps,
            tc.tile_pool(name="act", bufs=1) as actp,
        ):
            # --- constants ---
            wcs = constp.tile([128, 192], f32)
            nc.sync.dma_start(wcs[:], wc_d.ap()[:, :])
            cbb = constp.tile([128, 256], f32)
            nc.sync.dma_start(cbb[:], cb_d.ap()[:, :])
            idn = constp.tile([128, 128], f32)
            nc.sync.dma_start(idn[:], idn_d.ap()[:, :])
            b2s = constp.tile([1, HID], f32)
            nc.sync.dma_start(b2s[:], b2_d.ap()[:, :])
            b3s = constp.tile([1, OUT_DIM], f32)
            nc.sync.dma_start(b3s[:], b3_d.ap()[:, :])
            csts = constp.tile([128, 2], f32)
            nc.sync.dma_start(csts[:], cst_d.ap()[:, :])
            e511 = csts[:, 0:1]
            e0n = csts[:, 1:2]
            ones16 = constp.tile([1, 16], f32)
            nc.gpsimd.memset(ones16[:, :], 1.0)

            # --- persistent feature tensors ---
            ft = ftp.tile([128, NT1 * BPC], f32)       # [128, 576]
            ftr = ft[:].rearrange("p (t c) -> p t c", c=BPC)
            ft1ps = ftpsp.tile([64, BPC], f32)         # lvl1 staging

            # ======================= front-end =======================
            if True:
                for pair in range(BPC // 2):
                    xb = xbp.tile([128, 514], f32)
                    nc.gpsimd.memset(xb[:, 0:1], 0.0)
                    nc.gpsimd.memset(xb[:, 513:514], 0.0)
                    nc.sync.dma_start(xb[0:64, 1:513], xa[2 * pair, 0:C_IN, :])
                    nc.sync.dma_start(xb[64:128, 1:513], xa[2 * pair + 1, 0:C_IN, :])
                    for h in range(2):
                        b = 2 * pair + h
                        hb = 64 * h
                        # conv: out^T[l', o] accumulated over 3 taps
                        cv = cvpsp.tile([128, 256], f32)
                        for lt in range(4):
                            for k in range(3):
                                nc.tensor.matmul(
                                    cv[:, 64 * lt:64 * lt + 64],
                                    xb[hb:hb + 64, 128 * lt + k:128 * lt + k + 128],
                                    wcs[hb:hb + 64, 64 * k:64 * k + 64],
                                    start=(k == 0), stop=(k == 2),
                                )
                        # bias + tanh -> path tiles [l-part, 4 blocks x 64 chan]
                        tmp = tmpp.tile([128, 256], f32)
                        nc.vector.tensor_add(tmp[:, :], cv[:, :], cbb[:, :])
                        pt = ptp.tile([128, 256], f32)
                        nc.scalar.activation(pt[:, :], tmp[:, :], TANH)

                        # shifted copy (partition shift must go through DMA)
                        pts = ptshp.tile([128, 256], f32)
                        nc.sync.dma_start(pts[0:127, 0:256], pt[1:128, 0:256])
                        nc.sync.dma_start(pts[127:128, 0:192], pt[0:1, 64:256])
                        nc.sync.dma_start(pts[127:128, 192:256], pt[127:128, 192:256])
                        dd = ddp.tile([128, 256], f32)
                        nc.vector.tensor_sub(dd[:, :], pts[:, :], pt[:, :])

                        # p511 row staged to partition 0 for the rank-1 term
                        prow = prowp.tile([1, 64], f32)
                        nc.sync.dma_start(prow[0:1, 0:64], pt[127:128, 192:256])

                        # S''^T[j, m] = sum_t dd[t, j] pt[t, m] + p0[j] p511[m]
                        st = smallps.tile([128, 64], f32, tag="sm", name="st")[0:64, :]
                        for t in range(4):
                            nc.tensor.matmul(
                                st[:, :],
                                dd[:, 64 * t:64 * t + 64],
                                pt[:, 64 * t:64 * t + 64],
                                start=(t == 0), stop=False,
                            )
                        nc.tensor.matmul(
                            st[:, :], pt[0:1, 0:64], prow[0:1, 0:64],
                            start=False, stop=True,
                        )
                        # scatter S'' columns into FT (strided col copies)
                        nc.vector.tensor_copy(ftr[0:64, 0:32, b], st[0:64, 0:32])
                        nc.vector.tensor_copy(ftr[64:128, 0:32, b], st[0:64, 32:64])

                        # lvl1 column: path[511] - path[0]
                        nc.tensor.matmul(
                            ft1ps[0:64, b:b + 1], pt[:, 192:256], e511,
                            start=True, stop=False,
                        )
                        nc.tensor.matmul(
                            ft1ps[0:64, b:b + 1], pt[:, 0:64], e0n,
                            start=False, stop=True,
                        )

                # pooled: max over stream of channel C_IN
                xm = xmp.tile([BPC, 512], f32)
                nc.sync.dma_start(xm[:, :], xa[0:BPC, C_IN, :])
                pxm = xmp.tile([BPC, 1], f32)
                nc.vector.reduce_max(
                    pxm[:, :], xm[:, :], axis=bass.mybir.AxisListType.X)
                pxt = smallps.tile([128, 64], f32, tag="sm", name="pxt")
                nc.tensor.transpose(pxt[0:1, 0:BPC], pxm[:, :], idn[0:BPC, 0:BPC])
                nc.vector.tensor_copy(ft[0:1, 560:560 + BPC], pxt[0:1, 0:BPC])
                nc.vector.tensor_copy(ft[0:64, 512:512 + BPC], ft1ps[0:64, 0:BPC])

                # static features x[:, 65:448, 0] scattered into FT columns
                nc.sync.dma_start(
                    ft[64:128, 512:528],
                    xa[0:BPC, 65:129, 0:1].rearrange("b c o -> c (b o)"))
                nc.sync.dma_start(
                    ft[0:128, 528:544],
                    xa[0:BPC, 129:257, 0:1].rearrange("b c o -> c (b o)"))
                nc.sync.dma_start(
                    ft[0:128, 544:560],
                    xa[0:BPC, 257:385, 0:1].rearrange("b c o -> c (b o)"))
                nc.sync.dma_start(ft[1:128, 560:576], t35_d.ap()[:, :])
                nc.sync.dma_start(
                    ft[2:65, 560:576],
                    xa[0:BPC, 385:448, 0:1].rearrange("b c o -> c (b o)"))

            # ======================= MLP =======================
            if True:
                if W_BF16:
                    ftbf = actp.tile([128, NT1 * BPC], bf16)
                    nc.vector.tensor_copy(ftbf[:, :], ft[:, :])
                    ftmm = ftbf[:].rearrange("p (t c) -> p t c", c=BPC)
                else:
                    ftmm = ftr
                # fc1: H1[b, h] = FT.T @ W1T
                h1ps = [mlpps.tile([BPC, 512], f32, tag="hps", name=f"h1ps{i}")
                         for i in range(2)]
                for kt in range(NT1):
                    w1s = wsp.tile([128, HID], wdt, tag="ws")
                    nc.sync.dma_start(w1s[:, :], w1_d.ap()[128 * kt:128 * kt + 128, :])
                    for nt in range(2):
                        nc.tensor.matmul(
                            h1ps[nt][:, :],
                            mmdt(ftmm[:, kt, :]),
                            mmdt(w1s[:, 512 * nt:512 * nt + 512]),
                            start=(kt == 0), stop=(kt == NT1 - 1),
                        )
                h1 = actp.tile([BPC, HID], f32)
                for nt in range(2):
                    nc.vector.tensor_relu(h1[:, 512 * nt:512 * nt + 512], h1ps[nt][:, :])

                # transpose H1 -> H1T [128, 8*16]
                h1t = actp.tile([128, 128], wdt)
                for i in range(8):
                    tp = smallps.tile([128, 64], f32, tag="sm", name="tp")[:, 0:BPC]
                    nc.tensor.transpose(
                        tp[:, :], h1[:, 128 * i:128 * i + 128], idn[0:BPC, 0:BPC])
                    nc.vector.tensor_copy(h1t[:, 16 * i:16 * i + 16], tp[:, :])

                # fc2
                h2ps = [mlpps.tile([BPC, 512], f32, tag="hps", name=f"h2ps{i}")
                         for i in range(2)]
                for kt in range(8):
                    w2s = wsp.tile([128, HID], wdt, tag="ws")
                    nc.sync.dma_start(w2s[:, :], w2_d.ap()[128 * kt:128 * kt + 128, :])
                    for nt in range(2):
                        nc.tensor.matmul(
                            h2ps[nt][:, :],
                            mmdt(h1t[:, 16 * kt:16 * kt + 16]),
                            mmdt(w2s[:, 512 * nt:512 * nt + 512]),
                            start=(kt == 0), stop=False,
                        )
                for nt in range(2):
                    nc.tensor.matmul(
                        h2ps[nt][:, :], ones16[:, :], b2s[0:1, 512 * nt:512 * nt + 512],
                        start=False, stop=True,
                    )
                h2 = actp.tile([BPC, HID], f32)
                for nt in range(2):
                    nc.vector.tensor_relu(h2[:, 512 * nt:512 * nt + 512], h2ps[nt][:, :])

                h2t = actp.tile([128, 128], f32)
                for i in range(8):
                    tp = smallps.tile([128, 64], f32, tag="sm", name="tp")[:, 0:BPC]
                    nc.tensor.transpose(
                        tp[:, :], h2[:, 128 * i:128 * i + 128], idn[0:BPC, 0:BPC])
                    nc.vector.tensor_copy(h2t[:, 16 * i:16 * i + 16], tp[:, :])

                # fc3
                w3s = actp.tile([128, HID], f32)
                for kt in range(8):
                    nc.sync.dma_start(
                        w3s[:, OUT_DIM * kt:OUT_DIM * kt + OUT_DIM],
                        w3_d.ap()[128 * kt:128 * kt + 128, :])
                ops = mlpps.tile([BPC, 512], f32, tag="hps")
                for kt in range(8):
                    nc.tensor.matmul(
                        ops[:, 0:OUT_DIM],
                        h2t[:, 16 * kt:16 * kt + 16],
                        w3s[:, 128 * kt:128 * kt + 128],
                        start=(kt == 0), stop=False,
                    )
                nc.tensor.matmul(
                    ops[:, 0:OUT_DIM], ones16[:, :], b3s[:, :], start=False, stop=True)
                outsb = actp.tile([BPC, OUT_DIM], f32)
                nc.vector.tensor_copy(outsb[:, :], ops[:, 0:OUT_DIM])
                nc.sync.dma_start(outa[:, :], outsb[:, :])

    nc.compile()
    _prog_cache[key] = nc
    return nc


def _host_weights(conv_w, conv_b, fc1_w, fc1_b, fc2_w, fc2_b, fc3_w, fc3_b):
    f = np.float32
    conv_w = np.asarray(conv_w, f)
    wc = np.ascontiguousarray(
        np.tile(conv_w.transpose(1, 2, 0).reshape(64, 192), (2, 1)))
    cbb = np.ascontiguousarray(np.tile(np.asarray(conv_b, f)[None, :], (128, 4)))

    fc1_w = np.asarray(fc1_w, f)
    wfull = np.zeros((HID, 64, 64), f)
    iu, ju = np.triu_indices(64, 1)
    wtri = fc1_w[:, 64:2080]
    wfull[:, iu, ju] = 0.5 * wtri
    wfull[:, ju, iu] = -0.5 * wtri

    w1t = np.zeros((D1, HID), f)
    for t in range(32):
        w1t[128 * t:128 * t + 64, :] = wfull[:, t, :].T
        w1t[128 * t + 64:128 * t + 128, :] = wfull[:, 32 + t, :].T
    # tile 32: lvl1 (p<64), static chans 65..128 (p>=64)
    w1t[4096:4160, :] = fc1_w[:, 0:64].T
    w1t[4160:4224, :] = fc1_w[:, 2081:2145].T
    # tiles 33, 34: static chans 129..384
    w1t[4224:4352, :] = fc1_w[:, 2145:2273].T
    w1t[4352:4480, :] = fc1_w[:, 2273:2401].T
    # tile 35: p0 pooled, p1 const-1 -> fc1 bias, p2..64 static 385..447
    w1t[4480, :] = fc1_w[:, 2080]
    w1t[4481, :] = np.asarray(fc1_b, f)
    w1t[4482:4545, :] = fc1_w[:, 2401:2464].T

    if W_BF16:
        import ml_dtypes
        w1t = w1t.astype(ml_dtypes.bfloat16)
    w2t = np.ascontiguousarray(np.asarray(fc2_w, f).T)
    if W_BF16:
        import ml_dtypes
        w2t = w2t.astype(ml_dtypes.bfloat16)
    b2 = np.asarray(fc2_b, f)[None, :]
    w3t = np.ascontiguousarray(np.asarray(fc3_w, f).T)
    b3 = np.asarray(fc3_b, f)[None, :]
    idn = np.eye(128, dtype=f)
    csts = np.zeros((128, 2), f)
    csts[127, 0] = 1.0
    csts[0, 1] = -1.0
    t35c = np.zeros((127, BPC), f)
    t35c[0, :] = 1.0
    return dict(csts=csts, t35c=t35c, wconv=wc, cbias=cbb, w1t=w1t, w2t=w2t, b2=b2, w3t=w3t, b3=b3,
                idn=idn)


def make_in_maps(x, conv_w, conv_b, fc1_w, fc1_b, fc2_w, fc2_b, fc3_w, fc3_b):
    shared = _host_weights(conv_w, conv_b, fc1_w, fc1_b, fc2_w, fc2_b,
                           fc3_w, fc3_b)
    x = np.asarray(x, np.float32)
    in_maps = []
    for c in range(NCORES):
        m = dict(shared)
        m["x"] = np.ascontiguousarray(x[BPC * c:BPC * (c + 1)])
        in_maps.append(m)
    return in_maps


def kernel(x, conv_w, conv_b, fc1_w, fc1_b, fc2_w, fc2_b, fc3_w, fc3_b):
    from concourse.bass_utils import run_bass_kernel_spmd

    nc = _build_nc()
    in_maps = make_in_maps(x, conv_w, conv_b, fc1_w, fc1_b, fc2_w, fc2_b,
                           fc3_w, fc3_b)
    res = run_bass_kernel_spmd(nc, in_maps, list(range(NCORES)))
    out = np.concatenate([res.results[c]["out"] for c in range(NCORES)], axis=0)
    return out.astype(np.float32)
